# revision 8
# baseline (speedup 1.0000x reference)
"""Trainium2 Bass kernel for CrossViewContrast loss.

reference math (N=16384, D=128, tau=0.5):
    z1n = z1 / ||z1||,  z2n = z2 / ||z2||          (row-wise L2 norm)
    sim  = exp(z1n @ z2n.T / tau)                  # [N, N]
    pos  = exp(sum(z1n*z2n, -1) / tau)             # diag
    denom = sim.sum(1) + 1e-8
    loss = -mean(log(pos / denom))
         = mean( ln(denom_i) - 2 * z1n_i . z2n_i )

Sharding: rows of z1 split across 8 cores (2048 rows each); z2 replicated.
Each core emits per-row loss terms [128, 16]; host averages.

Per-core pipeline (ACT-engine bound, ~33.5M exp evals):
  - normalize z1 slice + z2 (1/||.|| = Exp(-0.5*Ln(sum sq)) -- stays in the
    natural_log_exp ACT table set, no table switching)
  - bf16 normalized copies, DMA-xbar transpose to [D, rows] layout
  - per (m-tile, 2048-col strip): 4 matmuls K=128 -> PSUM [128,2048] fp32,
    one ACT Exp(scale=2.0) over the PSUM tile with accum_out = row-sum
  - finish: denom = sum partials, ln(denom+1e-8) - 2*dot_ii, DMA out
"""

import os
from contextlib import ExitStack

import numpy as np

import concourse.bass as bass
import concourse.bacc as bacc
import concourse.tile as tile
from concourse import mybir

N, D = 16384, 128
NCORES = 8
R = N // NCORES          # rows of z1 per core = 2048
P = 128                  # partitions
MT = R // P              # m-tiles per core = 16
STRIP = 2048             # z2 columns handled per psum tile
NSTRIP = N // STRIP      # 8
ST = STRIP // P          # z2 row-tiles per strip = 16
MM_N = 512               # matmul moving free dim (one PSUM bank)
KCH = STRIP // MM_N      # matmuls per psum tile = 4
TAU_INV = 2.0            # 1/tau
EPS_DEN = 1e-8

F32 = mybir.dt.float32
BF16 = mybir.dt.bfloat16
AF = mybir.ActivationFunctionType
ALU = mybir.AluOpType


def _ttr_dot(nc, pool, a, b, accum):
    # row-dot: elementwise product then free-dim reduce (two standard DVE ops;
    # the fused TENSOR_TENSOR_REDUCE custom op fails walrus codegen here)
    sc = pool.tile([P, D], F32, tag="ttsc")
    nc.vector.tensor_mul(sc, a, b)
    nc.vector.reduce_sum(accum, sc, axis=mybir.AxisListType.X)


def _body(ctx, tc, z1s, z2, z2d, out_rows):
    nc = tc.nc

    # views: row j = tile*128 + p  ->  [p, tile, d]
    z1v = z1s.rearrange("(t p) d -> p t d", p=P)
    z2dv = z2d.rearrange("(t p) d -> p t d", p=P)
    z2v = z2.rearrange("(s u p) d -> s p u d", u=ST, p=P)

    singles = ctx.enter_context(tc.tile_pool(name="singles", bufs=1))
    stagep = ctx.enter_context(tc.tile_pool(name="stagep", bufs=3))
    nrmp = ctx.enter_context(tc.tile_pool(name="nrmp", bufs=4))
    stat = ctx.enter_context(tc.tile_pool(name="stat", bufs=3))
    z2tp = ctx.enter_context(tc.tile_pool(name="z2tp", bufs=2))
    expp = ctx.enter_context(tc.tile_pool(name="expp", bufs=2))
    psum = ctx.enter_context(
        tc.tile_pool(name="psum", bufs=2, space=bass.MemorySpace.PSUM)
    )

    partials = singles.tile([P, MT, NSTRIP], F32)
    ttsc = ctx.enter_context(tc.tile_pool(name="ttsc", bufs=2))
    zerob = singles.tile([P, 1], F32)
    nc.vector.memset(zerob, 0.0)
    epsb = singles.tile([P, 1], F32)
    nc.vector.memset(epsb, EPS_DEN)

    # ---------------- z1 slice: norms, normalize->bf16, transpose ----------
    z1st = singles.tile([P, MT, D], F32)
    nc.sync.dma_start(out=z1st, in_=z1v)
    ss1 = singles.tile([P, MT], F32)
    for t in range(MT):
        _ttr_dot(nc, ttsc, z1st[:, t, :], z1st[:, t, :], ss1[:, t : t + 1])
    rinv1 = singles.tile([P, MT], F32)
    nc.scalar.activation(rinv1, ss1, AF.Ln, bias=zerob)
    nc.scalar.activation(rinv1, rinv1, AF.Exp, bias=zerob, scale=-0.5)

    z1T = singles.tile([P, R], BF16)
    for t in range(MT):
        z1n = nrmp.tile([P, D], BF16, tag="nrm")
        nc.vector.tensor_scalar_mul(z1n, z1st[:, t, :], rinv1[:, t : t + 1])
        nc.sync.dma_start_transpose(z1T[:, t * P : (t + 1) * P], z1n)

    # ---------------- diagonal block of z2 (positives) ---------------------
    z2dst = singles.tile([P, MT, D], F32)
    nc.sync.dma_start(out=z2dst, in_=z2dv)
    ss2d = singles.tile([P, MT], F32)
    rawdot = singles.tile([P, MT], F32)
    for t in range(MT):
        _ttr_dot(nc, ttsc, z2dst[:, t, :], z2dst[:, t, :], ss2d[:, t : t + 1])
        _ttr_dot(nc, ttsc, z1st[:, t, :], z2dst[:, t, :], rawdot[:, t : t + 1])
    rinv2d = singles.tile([P, MT], F32)
    nc.scalar.activation(rinv2d, ss2d, AF.Ln, bias=zerob)
    nc.scalar.activation(rinv2d, rinv2d, AF.Exp, bias=zerob, scale=-0.5)

    # ---------------- main: per strip of 2048 z2 rows ----------------------
    for s in range(NSTRIP):
        stage = stagep.tile([P, ST, D], F32, tag="z2stage")
        nc.sync.dma_start(out=stage, in_=z2v[s])
        ss2 = stat.tile([P, ST], F32, tag="ss2")
        for u in range(ST):
            _ttr_dot(nc, ttsc, stage[:, u, :], stage[:, u, :], ss2[:, u : u + 1])
        rinv2 = stat.tile([P, ST], F32, tag="rinv2")
        nc.scalar.activation(rinv2, ss2, AF.Ln, bias=zerob)
        nc.scalar.activation(rinv2, rinv2, AF.Exp, bias=zerob, scale=-0.5)

        z2T = z2tp.tile([P, STRIP], BF16, tag="z2T")
        for u in range(ST):
            z2n = nrmp.tile([P, D], BF16, tag="nrm")
            nc.vector.tensor_scalar_mul(z2n, stage[:, u, :], rinv2[:, u : u + 1])
            nc.sync.dma_start_transpose(z2T[:, u * P : (u + 1) * P], z2n)

        for m in range(MT):
            ps = psum.tile([P, STRIP], F32, tag="mm")
            for k in range(KCH):
                nc.tensor.matmul(
                    ps[:, k * MM_N : (k + 1) * MM_N],
                    lhsT=z1T[:, m * P : (m + 1) * P],
                    rhs=z2T[:, k * MM_N : (k + 1) * MM_N],
                    start=True,
                    stop=True,
                )
            eo = expp.tile([P, STRIP], BF16, tag="expout")
            nc.scalar.activation(
                eo,
                ps,
                AF.Exp,
                bias=zerob,
                scale=TAU_INV,
                accum_out=partials[:, m, s : s + 1],
            )

    # ---------------- finish: per-row loss terms ----------------------------
    denom = singles.tile([P, MT, 1], F32)
    nc.vector.reduce_sum(denom, partials, axis=mybir.AxisListType.X)
    lnden = singles.tile([P, MT], F32)
    nc.scalar.activation(lnden, denom[:, :, 0], AF.Ln, bias=epsb)

    t1 = singles.tile([P, MT], F32)
    nc.vector.tensor_mul(t1, rawdot, rinv1)
    nc.vector.tensor_mul(t1, t1, rinv2d)
    nc.vector.tensor_scalar_mul(t1, t1, -2.0)
    res = singles.tile([P, MT], F32)
    nc.vector.tensor_add(res, lnden, t1)
    nc.sync.dma_start(out=out_rows, in_=res)


def build_program():
    nc = bacc.Bacc(None, target_bir_lowering=False)
    z1s = nc.declare_dram_parameter("z1s", [R, D], F32, isOutput=False)
    z2 = nc.declare_dram_parameter("z2", [N, D], F32, isOutput=False)
    z2d = nc.declare_dram_parameter("z2d", [R, D], F32, isOutput=False)
    out = nc.declare_dram_parameter("out_rows", [P, MT], F32, isOutput=True)

    with tile.TileContext(nc) as tc:
        with ExitStack() as ctx:
            _body(ctx, tc, z1s[:, :], z2[:, :], z2d[:, :], out[:, :])
    nc.finalize()
    return nc


_NC_CACHE = None


def _get_nc():
    global _NC_CACHE
    if _NC_CACHE is None:
        _NC_CACHE = build_program()
    return _NC_CACHE


def make_in_maps(z1, z2):
    z1 = np.ascontiguousarray(np.asarray(z1, dtype=np.float32))
    z2 = np.ascontiguousarray(np.asarray(z2, dtype=np.float32))
    in_maps = []
    for c in range(NCORES):
        sl = slice(c * R, (c + 1) * R)
        in_maps.append({"z1s": z1[sl].copy(), "z2": z2, "z2d": z2[sl].copy()})
    return in_maps


def kernel(z1, z2, _trace=False):
    from concourse.bass_utils import run_bass_kernel_spmd

    nc = _get_nc()
    in_maps = make_in_maps(z1, z2)
    res = run_bass_kernel_spmd(
        nc, in_maps, core_ids=list(range(NCORES)), trace=_trace
    )
    total = 0.0
    for r in res.results:
        total += np.asarray(r["out_rows"], dtype=np.float64).sum()
    loss = total / float(N)
    if _trace:
        return np.float32(loss), res
    return np.float32(loss)


if __name__ == "__main__":
    rng = np.random.default_rng(0)
    z1 = rng.standard_normal((N, D), dtype=np.float32)
    z2 = rng.standard_normal((N, D), dtype=np.float32)
    print(kernel(z1, z2))


# revision 9
# speedup vs baseline: 1.0684x; 1.0684x over previous
"""Trainium2 Bass kernel for CrossViewContrast loss.

reference math (N=16384, D=128, tau=0.5):
    z1n = z1 / ||z1||,  z2n = z2 / ||z2||          (row-wise L2 norm)
    sim  = exp(z1n @ z2n.T / tau)                  # [N, N]
    pos  = exp(sum(z1n*z2n, -1) / tau)             # diag
    denom = sim.sum(1) + 1e-8
    loss = -mean(log(pos / denom))
         = mean( ln(denom_i) - 2 * z1n_i . z2n_i )

Sharding: rows of z1 split across 8 cores (2048 rows each); z2 replicated.
Each core emits per-row loss terms [128, 16]; host averages.

Per-core pipeline (ACT-engine bound, ~33.5M exp evals):
  - normalize z1 slice + z2 (1/||.|| = Exp(-0.5*Ln(sum sq)) -- stays in the
    natural_log_exp ACT table set, no table switching)
  - bf16 normalized copies, DMA-xbar transpose to [D, rows] layout
  - per (m-tile, 2048-col strip): 4 matmuls K=128 -> PSUM [128,2048] fp32,
    one ACT Exp(scale=2.0) over the PSUM tile with accum_out = row-sum
  - finish: denom = sum partials, ln(denom+1e-8) - 2*dot_ii, DMA out
"""

import os
from contextlib import ExitStack

import numpy as np

import concourse.bass as bass
import concourse.bacc as bacc
import concourse.tile as tile
from concourse import mybir

N, D = 16384, 128
NCORES = 8
R = N // NCORES          # rows of z1 per core = 2048
P = 128                  # partitions
MT = R // P              # m-tiles per core = 16
STRIP = 2048             # z2 columns handled per psum tile
NSTRIP = N // STRIP      # 8
ST = STRIP // P          # z2 row-tiles per strip = 16
MM_N = 512               # matmul moving free dim (one PSUM bank)
KCH = STRIP // MM_N      # matmuls per psum tile = 4
TAU_INV = 2.0            # 1/tau
EPS_DEN = 1e-8

F32 = mybir.dt.float32
BF16 = mybir.dt.bfloat16
AF = mybir.ActivationFunctionType
ALU = mybir.AluOpType


def _patch_act_tables():
    """bacc's act-table-load pass greedily picks `natural_log` (5) for Ln and
    `exp_and_others` (0) for Exp, inserting a ~1.3us table load at every
    Ln<->Exp transition (17 loads in this kernel). Both functions live in
    `natural_log_exp_and_others` (6); hide Exp/Ln from every other set so the
    pass lands on 6 once. Index order is preserved (ids must stay aligned
    with act_info.json)."""
    import concourse.bacc as _bacc
    from concourse import mybir as _mb

    real = _bacc.get_activation_tables

    def patched(arch):
        tables = dict(real(arch))
        exp_ln = {_mb.ActivationFunctionType.Exp, _mb.ActivationFunctionType.Ln}
        joint = "natural_log_exp_and_others"
        if joint in tables and exp_ln <= set(tables[joint]):
            tables = {
                name: (set(fns) if name == joint else set(fns) - exp_ln)
                for name, fns in tables.items()
            }
        return tables

    _bacc.get_activation_tables = patched


_patch_act_tables()


def _ttr_dot(nc, pool, a, b, accum):
    # row-dot: elementwise product then free-dim reduce (two standard DVE ops;
    # the fused TENSOR_TENSOR_REDUCE custom op fails walrus codegen here)
    sc = pool.tile([P, D], F32, tag="ttsc")
    nc.vector.tensor_mul(sc, a, b)
    nc.vector.reduce_sum(accum, sc, axis=mybir.AxisListType.X)


def _body(ctx, tc, z1s, z2, z2d, out_rows):
    nc = tc.nc

    # views: row j = tile*128 + p  ->  [p, tile, d]
    z1v = z1s.rearrange("(t p) d -> p t d", p=P)
    z2dv = z2d.rearrange("(t p) d -> p t d", p=P)
    z2v = z2.rearrange("(s u p) d -> s p u d", u=ST, p=P)

    singles = ctx.enter_context(tc.tile_pool(name="singles", bufs=1))
    stagep = ctx.enter_context(tc.tile_pool(name="stagep", bufs=3))
    nrmp = ctx.enter_context(tc.tile_pool(name="nrmp", bufs=3))
    stat = ctx.enter_context(tc.tile_pool(name="stat", bufs=3))
    z2tp = ctx.enter_context(tc.tile_pool(name="z2tp", bufs=3))
    expp = ctx.enter_context(tc.tile_pool(name="expp", bufs=2))
    psum = ctx.enter_context(
        tc.tile_pool(name="psum", bufs=2, space=bass.MemorySpace.PSUM)
    )

    partials = singles.tile([P, MT, NSTRIP], F32)
    ttsc = ctx.enter_context(tc.tile_pool(name="ttsc", bufs=2))
    zerob = singles.tile([P, 1], F32)
    nc.vector.memset(zerob, 0.0)
    epsb = singles.tile([P, 1], F32)
    nc.vector.memset(epsb, EPS_DEN)

    # ---------------- z1 slice: norms, normalize->bf16, transpose ----------
    z1st = singles.tile([P, MT, D], F32)
    nc.gpsimd.dma_start(out=z1st, in_=z1v)
    ss1 = singles.tile([P, MT], F32)
    for t in range(MT):
        _ttr_dot(nc, ttsc, z1st[:, t, :], z1st[:, t, :], ss1[:, t : t + 1])
    rinv1 = singles.tile([P, MT], F32)
    nc.scalar.activation(rinv1, ss1, AF.Ln, bias=zerob)
    nc.scalar.activation(rinv1, rinv1, AF.Exp, bias=zerob, scale=-0.5)

    z1T = singles.tile([P, MT, P], BF16)
    z1n = singles.tile([P, MT * D], BF16)
    for t in range(MT):
        nc.vector.tensor_scalar_mul(
            z1n[:, t * D : (t + 1) * D], z1st[:, t, :], rinv1[:, t : t + 1]
        )
    nc.sync.dma_start_transpose(z1T, z1n)

    # ---------------- diagonal block of z2 (positives) ---------------------
    z2dst = singles.tile([P, MT, D], F32)
    nc.gpsimd.dma_start(out=z2dst, in_=z2dv)
    ss2d = singles.tile([P, MT], F32)
    rawdot = singles.tile([P, MT], F32)
    for t in range(MT):
        _ttr_dot(nc, ttsc, z2dst[:, t, :], z2dst[:, t, :], ss2d[:, t : t + 1])
        _ttr_dot(nc, ttsc, z1st[:, t, :], z2dst[:, t, :], rawdot[:, t : t + 1])
    rinv2d = singles.tile([P, MT], F32)
    nc.scalar.activation(rinv2d, ss2d, AF.Ln, bias=zerob)
    nc.scalar.activation(rinv2d, rinv2d, AF.Exp, bias=zerob, scale=-0.5)

    # ---------------- main: per strip of 2048 z2 rows ----------------------
    for s in range(NSTRIP):
        stage = stagep.tile([P, ST, D], F32, tag="z2stage")
        nc.gpsimd.dma_start(out=stage, in_=z2v[s])
        ss2 = stat.tile([P, ST], F32, tag="ss2")
        for u in range(ST):
            _ttr_dot(nc, ttsc, stage[:, u, :], stage[:, u, :], ss2[:, u : u + 1])
        rinv2 = stat.tile([P, ST], F32, tag="rinv2")
        nc.scalar.activation(rinv2, ss2, AF.Ln, bias=zerob)
        nc.scalar.activation(rinv2, rinv2, AF.Exp, bias=zerob, scale=-0.5)

        z2T = z2tp.tile([P, ST, P], BF16, tag="z2T")
        z2n = nrmp.tile([P, ST * D], BF16, tag="nrm")
        for u in range(ST):
            nc.vector.tensor_scalar_mul(
                z2n[:, u * D : (u + 1) * D], stage[:, u, :], rinv2[:, u : u + 1]
            )
        nc.sync.dma_start_transpose(z2T, z2n)

        for m in range(MT):
            ps = psum.tile([P, STRIP], F32, tag="mm")
            for k in range(KCH):
                nc.tensor.matmul(
                    ps[:, k * MM_N : (k + 1) * MM_N],
                    lhsT=z1T[:, m, :],
                    rhs=z2T.rearrange("p a b -> p (a b)")[
                        :, k * MM_N : (k + 1) * MM_N
                    ],
                    start=True,
                    stop=True,
                )
            eo = expp.tile([P, STRIP], BF16, tag="expout")
            nc.scalar.activation(
                eo,
                ps,
                AF.Exp,
                bias=zerob,
                scale=TAU_INV,
                accum_out=partials[:, m, s : s + 1],
            )

    # ---------------- finish: per-row loss terms ----------------------------
    denom = singles.tile([P, MT, 1], F32)
    nc.vector.reduce_sum(denom, partials, axis=mybir.AxisListType.X)
    lnden = singles.tile([P, MT], F32)
    nc.scalar.activation(lnden, denom[:, :, 0], AF.Ln, bias=epsb)

    t1 = singles.tile([P, MT], F32)
    nc.vector.tensor_mul(t1, rawdot, rinv1)
    nc.vector.tensor_mul(t1, t1, rinv2d)
    nc.vector.tensor_scalar_mul(t1, t1, -2.0)
    res = singles.tile([P, MT], F32)
    nc.vector.tensor_add(res, lnden, t1)
    nc.sync.dma_start(out=out_rows, in_=res)


def build_program():
    nc = bacc.Bacc(None, target_bir_lowering=False)
    z1s = nc.declare_dram_parameter("z1s", [R, D], F32, isOutput=False)
    z2 = nc.declare_dram_parameter("z2", [N, D], F32, isOutput=False)
    z2d = nc.declare_dram_parameter("z2d", [R, D], F32, isOutput=False)
    out = nc.declare_dram_parameter("out_rows", [P, MT], F32, isOutput=True)

    with tile.TileContext(nc) as tc:
        with ExitStack() as ctx:
            _body(ctx, tc, z1s[:, :], z2[:, :], z2d[:, :], out[:, :])
    nc.finalize()
    return nc


_NC_CACHE = None


def _get_nc():
    global _NC_CACHE
    if _NC_CACHE is None:
        _NC_CACHE = build_program()
    return _NC_CACHE


def make_in_maps(z1, z2):
    z1 = np.ascontiguousarray(np.asarray(z1, dtype=np.float32))
    z2 = np.ascontiguousarray(np.asarray(z2, dtype=np.float32))
    in_maps = []
    for c in range(NCORES):
        sl = slice(c * R, (c + 1) * R)
        in_maps.append({"z1s": z1[sl].copy(), "z2": z2, "z2d": z2[sl].copy()})
    return in_maps


def kernel(z1, z2, _trace=False):
    from concourse.bass_utils import run_bass_kernel_spmd

    nc = _get_nc()
    in_maps = make_in_maps(z1, z2)
    res = run_bass_kernel_spmd(
        nc, in_maps, core_ids=list(range(NCORES)), trace=_trace
    )
    total = 0.0
    for r in res.results:
        total += np.asarray(r["out_rows"], dtype=np.float64).sum()
    loss = total / float(N)
    if _trace:
        return np.float32(loss), res
    return np.float32(loss)


if __name__ == "__main__":
    rng = np.random.default_rng(0)
    z1 = rng.standard_normal((N, D), dtype=np.float32)
    z2 = rng.standard_normal((N, D), dtype=np.float32)
    print(kernel(z1, z2))


# revision 10
# speedup vs baseline: 1.1616x; 1.0872x over previous
"""Trainium2 Bass kernel for CrossViewContrast loss.

reference math (N=16384, D=128, tau=0.5):
    z1n = z1 / ||z1||,  z2n = z2 / ||z2||          (row-wise L2 norm)
    sim  = exp(z1n @ z2n.T / tau)                  # [N, N]
    pos  = exp(sum(z1n*z2n, -1) / tau)             # diag
    denom = sim.sum(1) + 1e-8
    loss = -mean(log(pos / denom))
         = mean( ln(denom_i) - 2 * z1n_i . z2n_i )

Sharding: rows of z1 split across 8 cores (2048 rows each); z2 replicated.
Each core emits per-row loss terms [128, 16]; host averages.

Per-core pipeline (ACT-engine bound, ~33.5M exp evals):
  - normalize z1 slice + z2 (1/||.|| = Exp(-0.5*Ln(sum sq)) -- stays in the
    natural_log_exp ACT table set, no table switching)
  - bf16 normalized copies, DMA-xbar transpose to [D, rows] layout
  - per (m-tile, 2048-col strip): 4 matmuls K=128 -> PSUM [128,2048] fp32,
    one ACT Exp(scale=2.0) over the PSUM tile with accum_out = row-sum
  - finish: denom = sum partials, ln(denom+1e-8) - 2*dot_ii, DMA out
"""

import os
from contextlib import ExitStack

import numpy as np

import concourse.bass as bass
import concourse.bacc as bacc
import concourse.tile as tile
from concourse import mybir

N, D = 16384, 128
NCORES = 8
R = N // NCORES          # rows of z1 per core = 2048
P = 128                  # partitions
MT = R // P              # m-tiles per core = 16
STRIP = 2048             # z2 columns handled per psum tile
NSTRIP = N // STRIP      # 8
ST = STRIP // P          # z2 row-tiles per strip = 16
MM_N = 512               # matmul moving free dim (one PSUM bank)
KCH = STRIP // MM_N      # matmuls per psum tile = 4
TAU_INV = 2.0            # 1/tau
EPS_DEN = 1e-8

F32 = mybir.dt.float32
BF16 = mybir.dt.bfloat16
AF = mybir.ActivationFunctionType
ALU = mybir.AluOpType


def _patch_act_tables():
    """bacc's act-table-load pass greedily picks `natural_log` (5) for Ln and
    `exp_and_others` (0) for Exp, inserting a ~1.3us table load at every
    Ln<->Exp transition (17 loads in this kernel). Both functions live in
    `natural_log_exp_and_others` (6); hide Exp/Ln from every other set so the
    pass lands on 6 once. Index order is preserved (ids must stay aligned
    with act_info.json)."""
    import concourse.bacc as _bacc
    from concourse import mybir as _mb

    real = _bacc.get_activation_tables

    def patched(arch):
        tables = dict(real(arch))
        exp_ln = {_mb.ActivationFunctionType.Exp, _mb.ActivationFunctionType.Ln}
        joint = "natural_log_exp_and_others"
        if joint in tables and exp_ln <= set(tables[joint]):
            tables = {
                name: (set(fns) if name == joint else set(fns) - exp_ln)
                for name, fns in tables.items()
            }
        return tables

    _bacc.get_activation_tables = patched


_patch_act_tables()


def _ttr_dot(nc, pool, a, b, accum):
    # row-dot: elementwise product then free-dim reduce (two standard DVE ops;
    # the fused TENSOR_TENSOR_REDUCE custom op fails walrus codegen here)
    sc = pool.tile([P, D], F32, tag="ttsc")
    nc.vector.tensor_mul(sc, a, b)
    nc.vector.reduce_sum(accum, sc, axis=mybir.AxisListType.X)


def _body(ctx, tc, z1s, z2, z2d, out_rows):
    nc = tc.nc

    # views: row j = tile*128 + p  ->  [p, tile, d]
    z1v = z1s.rearrange("(t p) d -> p t d", p=P)
    z2dv = z2d.rearrange("(t p) d -> p t d", p=P)
    z2v = z2.rearrange("(s u p) d -> s p u d", u=ST, p=P)

    singles = ctx.enter_context(tc.tile_pool(name="singles", bufs=1))
    stagep = ctx.enter_context(tc.tile_pool(name="stagep", bufs=8))
    nrmp = ctx.enter_context(tc.tile_pool(name="nrmp", bufs=4))
    stat = ctx.enter_context(tc.tile_pool(name="stat", bufs=3))
    z2tp = ctx.enter_context(tc.tile_pool(name="z2tp", bufs=8))
    expp = ctx.enter_context(tc.tile_pool(name="expp", bufs=2))
    psum = ctx.enter_context(
        tc.tile_pool(name="psum", bufs=2, space=bass.MemorySpace.PSUM)
    )

    partials = singles.tile([P, MT, NSTRIP], F32)
    ttsc = ctx.enter_context(tc.tile_pool(name="ttsc", bufs=2))
    zerob = singles.tile([P, 1], F32)
    nc.vector.memset(zerob, 0.0)
    epsb = singles.tile([P, 1], F32)
    nc.vector.memset(epsb, EPS_DEN)

    # ---------------- z1 slice: norms, normalize->bf16, transpose ----------
    z1st = singles.tile([P, MT, D], F32)
    nc.gpsimd.dma_start(out=z1st, in_=z1v)
    ss1 = singles.tile([P, MT], F32)
    for t in range(MT):
        _ttr_dot(nc, ttsc, z1st[:, t, :], z1st[:, t, :], ss1[:, t : t + 1])
    rinv1 = singles.tile([P, MT], F32)
    nc.scalar.activation(rinv1, ss1, AF.Ln, bias=zerob)
    nc.scalar.activation(rinv1, rinv1, AF.Exp, bias=zerob, scale=-0.5)

    z1T = singles.tile([P, MT, P], BF16)
    z1n = singles.tile([P, MT * D], BF16)
    for t in range(MT):
        nc.vector.tensor_scalar_mul(
            z1n[:, t * D : (t + 1) * D], z1st[:, t, :], rinv1[:, t : t + 1]
        )
    nc.sync.dma_start_transpose(z1T, z1n)

    # ---------------- diagonal block of z2 (positives) ---------------------
    z2dst = singles.tile([P, MT, D], F32)
    nc.gpsimd.dma_start(out=z2dst, in_=z2dv)
    ss2d = singles.tile([P, MT], F32)
    rawdot = singles.tile([P, MT], F32)
    for t in range(MT):
        _ttr_dot(nc, ttsc, z2dst[:, t, :], z2dst[:, t, :], ss2d[:, t : t + 1])
        _ttr_dot(nc, ttsc, z1st[:, t, :], z2dst[:, t, :], rawdot[:, t : t + 1])
    rinv2d = singles.tile([P, MT], F32)
    nc.scalar.activation(rinv2d, ss2d, AF.Ln, bias=zerob)
    nc.scalar.activation(rinv2d, rinv2d, AF.Exp, bias=zerob, scale=-0.5)

    # ---------------- main: per strip of 2048 z2 rows ----------------------
    for s in range(NSTRIP):
        stage = stagep.tile([P, ST, D], F32, tag="z2stage")
        nc.gpsimd.dma_start(out=stage, in_=z2v[s])
        ss2 = stat.tile([P, ST], F32, tag="ss2")
        for u in range(ST):
            _ttr_dot(nc, ttsc, stage[:, u, :], stage[:, u, :], ss2[:, u : u + 1])
        rinv2 = stat.tile([P, ST], F32, tag="rinv2")
        nc.scalar.activation(rinv2, ss2, AF.Ln, bias=zerob)
        nc.scalar.activation(rinv2, rinv2, AF.Exp, bias=zerob, scale=-0.5)

        z2T = z2tp.tile([P, ST, P], BF16, tag="z2T")
        z2n = nrmp.tile([P, ST * D], BF16, tag="nrm")
        for u in range(ST):
            nc.vector.tensor_scalar_mul(
                z2n[:, u * D : (u + 1) * D], stage[:, u, :], rinv2[:, u : u + 1]
            )
        nc.sync.dma_start_transpose(z2T, z2n)

        for m in range(MT):
            ps = psum.tile([P, STRIP], F32, tag="mm")
            for k in range(KCH):
                nc.tensor.matmul(
                    ps[:, k * MM_N : (k + 1) * MM_N],
                    lhsT=z1T[:, m, :],
                    rhs=z2T.rearrange("p a b -> p (a b)")[
                        :, k * MM_N : (k + 1) * MM_N
                    ],
                    start=True,
                    stop=True,
                )
            eo = expp.tile([P, STRIP], BF16, tag="expout")
            nc.scalar.activation(
                eo,
                ps,
                AF.Exp,
                bias=zerob,
                scale=TAU_INV,
                accum_out=partials[:, m, s : s + 1],
            )

    # ---------------- finish: per-row loss terms ----------------------------
    denom = singles.tile([P, MT, 1], F32)
    nc.vector.reduce_sum(denom, partials, axis=mybir.AxisListType.X)
    lnden = singles.tile([P, MT], F32)
    nc.scalar.activation(lnden, denom[:, :, 0], AF.Ln, bias=epsb)

    t1 = singles.tile([P, MT], F32)
    nc.vector.tensor_mul(t1, rawdot, rinv1)
    nc.vector.tensor_mul(t1, t1, rinv2d)
    nc.vector.tensor_scalar_mul(t1, t1, -2.0)
    res = singles.tile([P, MT], F32)
    nc.vector.tensor_add(res, lnden, t1)
    nc.sync.dma_start(out=out_rows, in_=res)


def build_program():
    nc = bacc.Bacc(None, target_bir_lowering=False)
    z1s = nc.declare_dram_parameter("z1s", [R, D], F32, isOutput=False)
    z2 = nc.declare_dram_parameter("z2", [N, D], F32, isOutput=False)
    z2d = nc.declare_dram_parameter("z2d", [R, D], F32, isOutput=False)
    out = nc.declare_dram_parameter("out_rows", [P, MT], F32, isOutput=True)

    with tile.TileContext(nc) as tc:
        with ExitStack() as ctx:
            _body(ctx, tc, z1s[:, :], z2[:, :], z2d[:, :], out[:, :])
    nc.finalize()
    return nc


_NC_CACHE = None


def _get_nc():
    global _NC_CACHE
    if _NC_CACHE is None:
        _NC_CACHE = build_program()
    return _NC_CACHE


def make_in_maps(z1, z2):
    z1 = np.ascontiguousarray(np.asarray(z1, dtype=np.float32))
    z2 = np.ascontiguousarray(np.asarray(z2, dtype=np.float32))
    in_maps = []
    for c in range(NCORES):
        sl = slice(c * R, (c + 1) * R)
        in_maps.append({"z1s": z1[sl].copy(), "z2": z2, "z2d": z2[sl].copy()})
    return in_maps


def kernel(z1, z2, _trace=False):
    from concourse.bass_utils import run_bass_kernel_spmd

    nc = _get_nc()
    in_maps = make_in_maps(z1, z2)
    res = run_bass_kernel_spmd(
        nc, in_maps, core_ids=list(range(NCORES)), trace=_trace
    )
    total = 0.0
    for r in res.results:
        total += np.asarray(r["out_rows"], dtype=np.float64).sum()
    loss = total / float(N)
    if _trace:
        return np.float32(loss), res
    return np.float32(loss)


if __name__ == "__main__":
    rng = np.random.default_rng(0)
    z1 = rng.standard_normal((N, D), dtype=np.float32)
    z2 = rng.standard_normal((N, D), dtype=np.float32)
    print(kernel(z1, z2))


# revision 11
# speedup vs baseline: 1.1709x; 1.0080x over previous
"""Trainium2 Bass kernel for CrossViewContrast loss.

reference math (N=16384, D=128, tau=0.5):
    z1n = z1 / ||z1||,  z2n = z2 / ||z2||          (row-wise L2 norm)
    sim  = exp(z1n @ z2n.T / tau)                  # [N, N]
    pos  = exp(sum(z1n*z2n, -1) / tau)             # diag
    denom = sim.sum(1) + 1e-8
    loss = -mean(log(pos / denom))
         = mean( ln(denom_i) - 2 * z1n_i . z2n_i )

Sharding: rows of z1 split across 8 cores (2048 rows each); z2 replicated.
Each core emits per-row loss terms [128, 16]; host averages.

Per-core pipeline (ACT-engine bound, ~33.5M exp evals):
  - normalize z1 slice + z2 (1/||.|| = Exp(-0.5*Ln(sum sq)) -- stays in the
    natural_log_exp ACT table set, no table switching)
  - bf16 normalized copies, DMA-xbar transpose to [D, rows] layout
  - per (m-tile, 2048-col strip): 4 matmuls K=128 -> PSUM [128,2048] fp32,
    one ACT Exp(scale=2.0) over the PSUM tile with accum_out = row-sum
  - finish: denom = sum partials, ln(denom+1e-8) - 2*dot_ii, DMA out
"""

import os
from contextlib import ExitStack

import numpy as np

import concourse.bass as bass
import concourse.bacc as bacc
import concourse.tile as tile
from concourse import mybir

N, D = 16384, 128
NCORES = 8
R = N // NCORES          # rows of z1 per core = 2048
P = 128                  # partitions
MT = R // P              # m-tiles per core = 16
STRIP = 2048             # z2 columns handled per psum tile
NSTRIP = N // STRIP      # 8
ST = STRIP // P          # z2 row-tiles per strip = 16
MM_N = 512               # matmul moving free dim (one PSUM bank)
KCH = STRIP // MM_N      # matmuls per psum tile = 4
TAU_INV = 2.0            # 1/tau
EPS_DEN = 1e-8

F32 = mybir.dt.float32
BF16 = mybir.dt.bfloat16
AF = mybir.ActivationFunctionType
ALU = mybir.AluOpType


def _patch_act_tables():
    """bacc's act-table-load pass greedily picks `natural_log` (5) for Ln and
    `exp_and_others` (0) for Exp, inserting a ~1.3us table load at every
    Ln<->Exp transition (17 loads in this kernel). Both functions live in
    `natural_log_exp_and_others` (6); hide Exp/Ln from every other set so the
    pass lands on 6 once. Index order is preserved (ids must stay aligned
    with act_info.json)."""
    import concourse.bacc as _bacc
    from concourse import mybir as _mb

    real = _bacc.get_activation_tables

    def patched(arch):
        tables = dict(real(arch))
        exp_ln = {_mb.ActivationFunctionType.Exp, _mb.ActivationFunctionType.Ln}
        joint = "natural_log_exp_and_others"
        if joint in tables and exp_ln <= set(tables[joint]):
            tables = {
                name: (set(fns) if name == joint else set(fns) - exp_ln)
                for name, fns in tables.items()
            }
        return tables

    _bacc.get_activation_tables = patched


_patch_act_tables()


def _ttr_dot(nc, pool, a, b, accum):
    # row-dot: elementwise product then free-dim reduce (two standard DVE ops;
    # the fused TENSOR_TENSOR_REDUCE custom op fails walrus codegen here)
    sc = pool.tile([P, D], F32, tag="ttsc")
    nc.vector.tensor_mul(sc, a, b)
    nc.vector.reduce_sum(accum, sc, axis=mybir.AxisListType.X)


def _body(ctx, tc, z1s, z2, z2d, out_rows):
    nc = tc.nc

    # views: row j = tile*128 + p  ->  [p, tile, d]
    z1v = z1s.rearrange("(t p) d -> p t d", p=P)
    z2dv = z2d.rearrange("(t p) d -> p t d", p=P)
    z2v = z2.rearrange("(s u p) d -> s p u d", u=ST, p=P)

    singles = ctx.enter_context(tc.tile_pool(name="singles", bufs=1))
    stagep = ctx.enter_context(tc.tile_pool(name="stagep", bufs=8))
    nrmp = ctx.enter_context(tc.tile_pool(name="nrmp", bufs=4))
    stat = ctx.enter_context(tc.tile_pool(name="stat", bufs=3))
    z2tp = ctx.enter_context(tc.tile_pool(name="z2tp", bufs=8))
    expp = ctx.enter_context(tc.tile_pool(name="expp", bufs=2))
    psum = ctx.enter_context(
        tc.tile_pool(name="psum", bufs=2, space=bass.MemorySpace.PSUM)
    )

    partials = singles.tile([P, MT, NSTRIP], F32)
    ttsc = ctx.enter_context(tc.tile_pool(name="ttsc", bufs=2))
    zerob = singles.tile([P, 1], F32)
    nc.vector.memset(zerob, 0.0)
    epsb = singles.tile([P, 1], F32)
    nc.vector.memset(epsb, EPS_DEN)

    # ---------------- z1 slice: norms, normalize->bf16, transpose ----------
    z1st = singles.tile([P, MT, D], F32)
    nc.gpsimd.dma_start(out=z1st, in_=z1v)
    ss1 = singles.tile([P, MT], F32)
    for t in range(MT):
        _ttr_dot(nc, ttsc, z1st[:, t, :], z1st[:, t, :], ss1[:, t : t + 1])
    rinv1 = singles.tile([P, MT], F32)
    nc.scalar.activation(rinv1, ss1, AF.Ln, bias=zerob)
    nc.scalar.activation(rinv1, rinv1, AF.Exp, bias=zerob, scale=-0.5)

    z1T = singles.tile([P, MT, P], BF16)
    z1n = singles.tile([P, MT * D], BF16)
    for t in range(MT):
        nc.vector.tensor_scalar_mul(
            z1n[:, t * D : (t + 1) * D], z1st[:, t, :], rinv1[:, t : t + 1]
        )
    nc.sync.dma_start_transpose(z1T, z1n)

    # ---------------- main: per strip of 2048 z2 rows ----------------------
    for s in range(NSTRIP):
        stage = stagep.tile([P, ST, D], F32, tag="z2stage")
        nc.gpsimd.dma_start(out=stage, in_=z2v[s])
        ss2 = stat.tile([P, ST], F32, tag="ss2")
        for u in range(ST):
            _ttr_dot(nc, ttsc, stage[:, u, :], stage[:, u, :], ss2[:, u : u + 1])
        rinv2 = stat.tile([P, ST], F32, tag="rinv2")
        nc.scalar.activation(rinv2, ss2, AF.Ln, bias=zerob)
        nc.scalar.activation(rinv2, rinv2, AF.Exp, bias=zerob, scale=-0.5)

        z2T = z2tp.tile([P, ST, P], BF16, tag="z2T")
        z2n = nrmp.tile([P, ST * D], BF16, tag="nrm")
        for u in range(ST):
            nc.vector.tensor_scalar_mul(
                z2n[:, u * D : (u + 1) * D], stage[:, u, :], rinv2[:, u : u + 1]
            )
        nc.sync.dma_start_transpose(z2T, z2n)

        for m in range(MT):
            ps = psum.tile([P, STRIP], F32, tag="mm")
            for k in range(KCH):
                nc.tensor.matmul(
                    ps[:, k * MM_N : (k + 1) * MM_N],
                    lhsT=z1T[:, m, :],
                    rhs=z2T.rearrange("p a b -> p (a b)")[
                        :, k * MM_N : (k + 1) * MM_N
                    ],
                    start=True,
                    stop=True,
                )
            eo = expp.tile([P, STRIP], BF16, tag="expout")
            nc.scalar.activation(
                eo,
                ps,
                AF.Exp,
                bias=zerob,
                scale=TAU_INV,
                accum_out=partials[:, m, s : s + 1],
            )

    # ---------------- diagonal block of z2 (positives) ---------------------
    z2dst = singles.tile([P, MT, D], F32)
    nc.gpsimd.dma_start(out=z2dst, in_=z2dv)
    ss2d = singles.tile([P, MT], F32)
    rawdot = singles.tile([P, MT], F32)
    for t in range(MT):
        _ttr_dot(nc, ttsc, z2dst[:, t, :], z2dst[:, t, :], ss2d[:, t : t + 1])
        _ttr_dot(nc, ttsc, z1st[:, t, :], z2dst[:, t, :], rawdot[:, t : t + 1])
    rinv2d = singles.tile([P, MT], F32)
    nc.scalar.activation(rinv2d, ss2d, AF.Ln, bias=zerob)
    nc.scalar.activation(rinv2d, rinv2d, AF.Exp, bias=zerob, scale=-0.5)

    # ---------------- finish: per-row loss terms ----------------------------
    denom = singles.tile([P, MT, 1], F32)
    nc.vector.reduce_sum(denom, partials, axis=mybir.AxisListType.X)
    lnden = singles.tile([P, MT], F32)
    nc.scalar.activation(lnden, denom[:, :, 0], AF.Ln, bias=epsb)

    t1 = singles.tile([P, MT], F32)
    nc.vector.tensor_mul(t1, rawdot, rinv1)
    nc.vector.tensor_mul(t1, t1, rinv2d)
    nc.vector.tensor_scalar_mul(t1, t1, -2.0)
    res = singles.tile([P, MT], F32)
    nc.vector.tensor_add(res, lnden, t1)
    nc.sync.dma_start(out=out_rows, in_=res)


def build_program():
    nc = bacc.Bacc(None, target_bir_lowering=False)
    z1s = nc.declare_dram_parameter("z1s", [R, D], F32, isOutput=False)
    z2 = nc.declare_dram_parameter("z2", [N, D], F32, isOutput=False)
    z2d = nc.declare_dram_parameter("z2d", [R, D], F32, isOutput=False)
    out = nc.declare_dram_parameter("out_rows", [P, MT], F32, isOutput=True)

    with tile.TileContext(nc) as tc:
        with ExitStack() as ctx:
            _body(ctx, tc, z1s[:, :], z2[:, :], z2d[:, :], out[:, :])
    nc.finalize()
    return nc


_NC_CACHE = None


def _get_nc():
    global _NC_CACHE
    if _NC_CACHE is None:
        _NC_CACHE = build_program()
    return _NC_CACHE


def make_in_maps(z1, z2):
    z1 = np.ascontiguousarray(np.asarray(z1, dtype=np.float32))
    z2 = np.ascontiguousarray(np.asarray(z2, dtype=np.float32))
    in_maps = []
    for c in range(NCORES):
        sl = slice(c * R, (c + 1) * R)
        in_maps.append({"z1s": z1[sl].copy(), "z2": z2, "z2d": z2[sl].copy()})
    return in_maps


def kernel(z1, z2, _trace=False):
    from concourse.bass_utils import run_bass_kernel_spmd

    nc = _get_nc()
    in_maps = make_in_maps(z1, z2)
    res = run_bass_kernel_spmd(
        nc, in_maps, core_ids=list(range(NCORES)), trace=_trace
    )
    total = 0.0
    for r in res.results:
        total += np.asarray(r["out_rows"], dtype=np.float64).sum()
    loss = total / float(N)
    if _trace:
        return np.float32(loss), res
    return np.float32(loss)


if __name__ == "__main__":
    rng = np.random.default_rng(0)
    z1 = rng.standard_normal((N, D), dtype=np.float32)
    z2 = rng.standard_normal((N, D), dtype=np.float32)
    print(kernel(z1, z2))
